# revision 46
# baseline (speedup 1.0000x reference)
"""DigitCapsule (dynamic routing) Trainium2 Bass kernel.

Problem: x (128,1152,8) f32, W (1,1152,10,16,8) f32 ->
  u_hat[b,r,o,do] = sum_di W[r,o,do,di] x[b,r,di]
  3 routing iterations (softmax over routes r, squash), output v (128,10,16,1).

Sharding: data-parallel over batch, 16 samples per core, W replicated.

Per-core layout (partition p = 16*j + b, j = r mod 8, b = batch-in-core):
  u[p, cc, do, o] = u_hat[b, 8*cc+j, o, do]   (fp16, 144 x 16 x 10 free)

Key structure:
  - xd (block-diag x stationary) built on host incl. zeros -> plain DMA
    (input DMA 5.4 MB total; the DMA stream paces the production phase).
  - u produced by 144 matmuls; PSUM->SBUF eviction alternates DVE/Act.
  - s0 = sum_r u via the delta-matrix chain (d16) interleaved into the
    production stream with a 2-batch lag (PE is in-order).
  - squash is elementwise: v = s*|s|/(1+s^2)  (mag_sq in the reference is
    over the trailing singleton axis).  Only Exp/Abs/Square activation
    functions are used -> a single LoadActFuncSet.
  - agreement premul+tree all-fp16 (2x DVE mode); the Pool engine owns
    group 0's premul+tree end to end (own tile buffers so it never blocks
    the in-order DVE stream).
  - exp per group on Act, overlapped with the agreement trees.
  - s-chain: c16 and premuls on DVE (Pool does group 0's premul); the
    psum accumulation consumes Pool's group mid-chain; dummy matmuls
    pre-warm the PE p-state during the softmax window.
  - input DMAs are chunked and emitted interleaved with the production
    batches (DMA completion sems are cumulative per queue, so emitting
    them all up front would gate the first matmul on the last chunk).
"""

import numpy as np

import concourse.bacc as bacc
import concourse.bass as bass
import concourse.tile as tile
from concourse import mybir
from concourse.bass_utils import run_bass_kernel_spmd

B, R, O, DO, DI = 128, 1152, 10, 16, 8
NCORES = 8
BC = B // NCORES          # 16 samples per core
J = 8                     # routes per matmul group
CC = R // J               # 144 matmul groups
OD = O * DO               # 160
G0 = 72                   # k=128 chunks for direct s0 (16 routes x 8 di)
F16 = mybir.dt.float16
F32 = mybir.dt.float32
AF = mybir.ActivationFunctionType
ALU = mybir.AluOpType

PROD_BATCH = 2            # cc per production psum batch (1 bank each)
TREE_BATCH = 24           # cc per premult/tree batch
NG = CC // TREE_BATCH     # 6 groups
POOL_GROUP = 0            # premul/tree group owned by the Pool engine
N_WARM = 3                # PE warm-up dummy matmuls per routing iteration


def _tl(pool, shape, tag):
    tile_h = pool.tile(shape, F16, tag=tag, name=tag)
    return tile_h


def _squash_elem(nc, pool, s_ps, v_out, scale, tag):
    """v_out = squash(s_ps * scale) elementwise: v = k2*s*|s| / (1 + (k*s)^2)."""
    P = s_ps.shape[0]
    q = pool.tile([P, DO, O], F32, tag=tag + "q")
    ab = pool.tile([P, DO, O], F32, tag=tag + "a")
    d = pool.tile([P, DO, O], F32, tag=tag + "d")
    p1 = pool.tile([P, DO, O], F32, tag=tag + "p")
    nc.scalar.activation(q[:], s_ps[:], AF.Square, scale=float(scale))
    nc.scalar.activation(ab[:], s_ps[:], AF.Abs, scale=float(scale * scale))
    nc.vector.tensor_scalar_add(d[:], q[:], 1.0)
    nc.vector.reciprocal(d[:], d[:])
    nc.vector.tensor_mul(p1[:], s_ps[:], ab[:])
    nc.vector.tensor_mul(v_out[:], p1[:], d[:])


def build_nc():
    nc = bacc.Bacc("TRN2", debug=False)
    wt_d = nc.dram_tensor("wt", [64, CC, DO, O], F16, kind="ExternalInput")
    xd_d = nc.dram_tensor("xd", [64, CC, 128], F16, kind="ExternalInput")
    xs_d = nc.dram_tensor("xs", [128, 72, BC], F16, kind="ExternalInput")
    ws_d = nc.dram_tensor("ws", [128, 72, DO, O], F16, kind="ExternalInput")
    bc16_d = nc.dram_tensor("bc16", [BC, 128], F16, kind="ExternalInput")
    d16_d = nc.dram_tensor("d16", [128, 128], F16, kind="ExternalInput")
    d32_d = nc.dram_tensor("d32", [128, 128], F32, kind="ExternalInput")
    dout_d = nc.dram_tensor("dout", [128, BC], F16, kind="ExternalInput")
    out_d = nc.dram_tensor("out", [BC, O, DO], F32, kind="ExternalOutput")

    with tile.TileContext(nc) as tc:
        with (
            tc.tile_pool(name="const", bufs=1) as const,
            tc.tile_pool(name="prod", bufs=1) as prod,
            tc.tile_pool(name="main", bufs=1) as main,
            tc.tile_pool(name="sq", bufs=1) as sq,
            tc.tile_pool(name="tp", bufs=3) as tp,
            tc.tile_pool(name="l1p", bufs=2) as l1p,
            tc.tile_pool(name="l2p", bufs=2) as l2p,
            tc.tile_pool(name="l3p", bufs=2) as l3p,
            tc.tile_pool(name="l4p", bufs=2) as l4p,
            tc.tile_pool(name="pb", bufs=1) as pb,
            tc.tile_pool(name="pp", bufs=3, space=bass.MemorySpace.PSUM) as pp,
            tc.tile_pool(name="pss", bufs=1, space=bass.MemorySpace.PSUM) as pss,
            tc.tile_pool(name="psd", bufs=1, space=bass.MemorySpace.PSUM) as psd,
        ):
            d16 = const.tile([128, 128], F16)
            d32 = const.tile([128, 128], F32)
            dout = const.tile([128, BC], F16)
            bc16 = const.tile([BC, 128], F16)
            nc.sync.dma_start(d16[:], d16_d[:])
            nc.sync.dma_start(d32[:], d32_d[:])
            nc.sync.dma_start(dout[:], dout_d[:])
            nc.sync.dma_start(bc16[:], bc16_d[:])
            zero = const.tile([128, 1], F32)
            ebias = const.tile([128, 1], F32)
            nc.vector.memset(zero[:], 0.0)
            nc.vector.memset(ebias[:], -13.8629436)

            NCH = 8
            cch = CC // NCH
            g0pc = G0 // NCH              # s0 k-chunks per DMA chunk (9)
            xd_t, wt_t, ws_t = [None] * NCH, [None] * NCH, [None] * NCH

            xs = prod.tile([128, G0, BC], F16)

            def fetch_chunk(ch):
                sl = slice(ch * cch, (ch + 1) * cch)
                xd_c = prod.tile([64, cch, 128], F16, tag=f"xd{ch % 3}",
                                 name="xd_c")
                wt_c = prod.tile([64, cch, DO, O], F16, tag=f"wt{ch % 3}",
                                 name="wt_c")
                ws_c = prod.tile([128, g0pc, DO, O], F16, tag=f"ws{ch % 3}",
                                 name="ws_c")
                sg = slice(ch * g0pc, (ch + 1) * g0pc)
                nc.sync.dma_start(ws_c[:], ws_d[:, sg, :, :])
                nc.sync.dma_start(xd_c[:], xd_d[:, sl, :])
                nc.sync.dma_start(wt_c[:], wt_d[:, sl, :, :])
                xd_t[ch] = xd_c
                wt_t[ch] = wt_c
                ws_t[ch] = ws_c

            fetch_chunk(0)
            nc.sync.dma_start(xs[:], xs_d[:])
            fetch_chunk(1)

            u = main.tile([128, CC, DO, O], F16)

            # ---- produce u_hat; s0 accumulates directly from (x, W) ----
            s0_ps = pss.tile([BC, DO, O], F32, tag="s")
            nb = CC // PROD_BATCH
            bpc = cch // PROD_BATCH       # batches per DMA chunk (9)
            v = main.tile([128, DO, O], F16)

            def emit_s0_chunk(cs):
                for k in range(g0pc):
                    g0 = cs * g0pc + k
                    nc.tensor.matmul(
                        s0_ps[:], xs[:, g0, :], ws_t[cs][:, k, :, :],
                        start=(g0 == 0), stop=(g0 == G0 - 1),
                    )

            for g in range(nb):
                ch = g // bpc
                if g % bpc == 0:
                    # prefetch two chunks ahead of consumption
                    if ch + 2 < NCH:
                        fetch_chunk(ch + 2)
                    # s0 matmuls run one segment ahead of their chunk's
                    # production so s0 (hence v0) completes before the
                    # eviction-paced production tail
                    if ch == 0:
                        emit_s0_chunk(0)
                        emit_s0_chunk(1)
                    elif ch + 1 < NCH:
                        emit_s0_chunk(ch + 1)
                    if ch == 6:
                        # v0 = squash(s0 / R) on 16 partitions + broadcast.
                        # All on DVE: the Act queue is full of tail
                        # evictions, and DVE is idle waiting for v anyway.
                        s0sb = sq.tile([BC, DO, O], F32, tag="s0s")
                        q0 = sq.tile([BC, DO, O], F32, tag="s0q")
                        a0 = sq.tile([BC, DO, O], F32, tag="s0a")
                        v0_16 = main.tile([BC, DO, O], F16)
                        k = 1.0 / R
                        nc.vector.tensor_scalar_mul(s0sb[:], s0_ps[:], k)
                        nc.vector.tensor_mul(q0[:], s0sb[:], s0sb[:])
                        nc.vector.scalar_tensor_tensor(
                            a0[:], s0sb[:], -1.0, s0sb[:],
                            ALU.mult, ALU.max)
                        nc.vector.tensor_scalar_add(q0[:], q0[:], 1.0)
                        nc.vector.reciprocal(q0[:], q0[:])
                        nc.vector.tensor_mul(a0[:], a0[:], s0sb[:])
                        nc.vector.tensor_mul(a0[:], a0[:], q0[:])
                        nc.vector.tensor_copy(v0_16[:], a0[:])
                        v_ps = pss.tile([128, DO, O], F32, tag="s")
                        nc.tensor.matmul(v_ps[:], bc16[:], v0_16[:],
                                         start=True, stop=True)
                        nc.vector.tensor_copy(v[:], v_ps[:])
                ps = pp.tile([128, PROD_BATCH, 512], F32, tag="pp")
                for i in range(PROD_BATCH):
                    cc = g * PROD_BATCH + i
                    ci = cc % cch
                    nc.tensor.matmul(
                        ps[:, i, 0:OD], xd_t[ch][:, ci, :],
                        wt_t[ch][:, ci, :, :],
                        start=True, stop=True,
                    )
                sl = slice(g * PROD_BATCH, (g + 1) * PROD_BATCH)
                src = ps[:, :, 0:OD].rearrange(
                    "p c (do o) -> p c do o", do=DO)
                # first 6 chunks: Act and DVE each copy one cc of the batch
                # concurrently; tail on Act only so DVE is free to start
                # the iter-1 agreement as soon as v0 lands
                if ch >= 6:
                    nc.scalar.copy(u[:, sl, :, :], src)
                else:
                    cc0 = g * PROD_BATCH
                    nc.scalar.copy(u[:, cc0, :, :], src[:, 0, :, :])
                    nc.vector.tensor_copy(u[:, cc0 + 1, :, :], src[:, 1, :, :])

            b_ij = main.tile([128, CC, O], F32)
            e16 = main.tile([128, CC, O], F16)
            escr = main.tile([128, CC], F16)
            e_r = main.tile([128, O], F32)
            inv = main.tile([128, O], F32)

            for it in (1, 2):
                final = it == 2
                # ---- agreement: b_ij (+)= sum_do u * v ----
                l3_last = None
                for g in range(NG):
                    sl = slice(g * TREE_BATCH, (g + 1) * TREE_BATCH)
                    pool_g = g == POOL_GROUP
                    eng = nc.gpsimd if pool_g else nc.vector
                    sfx = "P" if pool_g else ""
                    if pool_g:
                        t = pb.tile([128, TREE_BATCH, DO, O], F16,
                                    tag="t0P", name="t")
                    else:
                        t = tp.tile([128, TREE_BATCH, DO, O], F16, tag="t")
                    v_b = v[:].unsqueeze(1).broadcast_to((128, TREE_BATCH, DO, O))
                    eng.tensor_mul(t[:], u[:, sl, :, :], v_b)
                    l1 = _tl(pb if pool_g else l1p, [128, TREE_BATCH, 8, O], "l1" + sfx)
                    eng.tensor_add(l1[:], t[:, :, 0:8, :], t[:, :, 8:16, :])
                    l2 = _tl(pb if pool_g else l2p, [128, TREE_BATCH, 4, O], "l2" + sfx)
                    eng.tensor_add(l2[:], l1[:, :, 0:4, :], l1[:, :, 4:8, :])
                    l3 = _tl(pb if pool_g else l3p, [128, TREE_BATCH, 2, O], "l3" + sfx)
                    eng.tensor_add(l3[:], l2[:, :, 0:2, :], l2[:, :, 2:4, :])
                    if it == 1:
                        eng.tensor_add(
                            b_ij[:, sl, :], l3[:, :, 0, :], l3[:, :, 1, :])
                    else:
                        a4 = _tl(pb if pool_g else l4p, [128, TREE_BATCH, O], "l4" + sfx)
                        eng.tensor_add(a4[:], l3[:, :, 0, :], l3[:, :, 1, :])
                        eng.tensor_add(b_ij[:, sl, :], b_ij[:, sl, :], a4[:])
                    if not pool_g:
                        l3_last = l3
                    # exp of this group overlaps the next group's tree
                    # (Act).  fp16 with a fixed bias shift: b1 in [-3, 3],
                    # b2 max ~24 -> shift by 20*ln2 keeps it in fp16 range;
                    # the softmax normalizer absorbs the shift exactly.
                    bias = zero if it == 1 else ebias
                    nc.scalar.activation(e16[:, sl, :], b_ij[:, sl, :],
                                         AF.Exp, bias=bias[:])

                # PE p-state warm-up while the premuls restart the stream
                warm = psd.tile([128, 512], F32, tag="den")
                for w in range(N_WARM):
                    nc.tensor.matmul(
                        warm[:, 0:480],
                        d16[:], l3_last[:].rearrange("p c x o -> p (c x o)"),
                        start=True, stop=True)

                # ---- s_raw = sum_r e * u  (normalizer applied post-hoc,
                # so premuls don't wait for the denominator) ----
                sp_p = BC if final else 128
                lhs = dout if final else d16
                s_ps2 = pss.tile([sp_p, DO, O], F32, tag="s")
                order = [POOL_GROUP] + [g for g in range(NG) if g != POOL_GROUP]
                mm_order = [1, 2, 3, POOL_GROUP, 4, 5]
                t_tiles = {}
                for k, g in enumerate(order):
                    sl = slice(g * TREE_BATCH, (g + 1) * TREE_BATCH)
                    pool_g = g == POOL_GROUP
                    eng = nc.gpsimd if pool_g else nc.vector
                    if pool_g:
                        t = pb.tile([128, TREE_BATCH, DO, O], F16, tag="t0P")
                    else:
                        t = tp.tile([128, TREE_BATCH, DO, O], F16, tag="t")
                    c_b = e16[:, sl, :].unsqueeze(2).broadcast_to(
                        (128, TREE_BATCH, DO, O))
                    if g == NG - 1:
                        # halves: matmuls on the first half overlap the
                        # second half's premul, shrinking the phase tail
                        hb = TREE_BATCH // 2
                        c_b1 = e16[:, sl, :][:, 0:hb, :].unsqueeze(2).broadcast_to(
                            (128, hb, DO, O))
                        c_b2 = e16[:, sl, :][:, hb:, :].unsqueeze(2).broadcast_to(
                            (128, hb, DO, O))
                        eng.tensor_mul(t[:, 0:hb, :, :],
                                       u[:, sl, :, :][:, 0:hb, :, :], c_b1)
                        for i in range(hb):
                            nc.tensor.matmul(
                                s_ps2[:], lhs[:, :sp_p], t[:, i, :, :],
                                start=False, stop=False)
                        eng.tensor_mul(t[:, hb:, :, :],
                                       u[:, sl, :, :][:, hb:, :, :], c_b2)
                        for i in range(hb, TREE_BATCH):
                            nc.tensor.matmul(
                                s_ps2[:], lhs[:, :sp_p], t[:, i, :, :],
                                start=False, stop=(i == TREE_BATCH - 1))
                        t_tiles[g] = t
                        continue
                    eng.tensor_mul(t[:], u[:, sl, :, :], c_b)
                    t_tiles[g] = t
                    if pool_g:
                        continue
                    ki = mm_order.index(g)
                    for i in range(TREE_BATCH):
                        nc.tensor.matmul(
                            s_ps2[:], lhs[:, :sp_p], t[:, i, :, :],
                            start=(ki == 0 and i == 0),
                            stop=False,
                        )
                    if ki + 1 < NG and mm_order[ki + 1] == POOL_GROUP:
                        tpg = t_tiles[POOL_GROUP]
                        for i in range(TREE_BATCH):
                            nc.tensor.matmul(
                                s_ps2[:], lhs[:, :sp_p], tpg[:, i, :, :],
                                start=False, stop=False,
                            )
                # ---- softmax denominator: 10 per-o accumulate-copies on
                # the otherwise-idle Act engine (frees DVE of the reduce) ----
                for o in range(O):
                    nc.scalar.activation(
                        escr[:], e16[:, :, o], AF.Copy,
                        accum_out=e_r[:, o : o + 1])
                den = psd.tile([128, 512], F32, tag="den")
                nc.tensor.matmul(den[:, 0:O], d32[:], e_r[:], start=True, stop=True)
                nc.vector.reciprocal(inv[:], den[:, 0:O])
                # ---- s = s_raw * inv, then squash ----
                sx = sq.tile([sp_p, DO, O], F32, tag="sx", name="sx")
                ivb = inv[0:sp_p, :].unsqueeze(1).broadcast_to((sp_p, DO, O))
                nc.vector.tensor_mul(sx[:], s_ps2[:], ivb)
                if not final:
                    _squash_elem(nc, sq, sx, v, 1.0, tag="sv")
                else:
                    v2 = main.tile([BC, DO, O], F32)
                    _squash_elem(nc, sq, sx, v2, 1.0, tag="sf")
                    v2p = main.tile([BC, O, DO], F32)
                    nc.vector.tensor_copy(v2p[:], v2[:].transpose((0, 2, 1)))
                    nc.sync.dma_start(out_d[:], v2p[:])

    nc.compile()
    return nc


_CACHE = {}


def _get_nc():
    if "nc" not in _CACHE:
        _CACHE["nc"] = build_nc()
    return _CACHE["nc"]


def _prep_const():
    if "const" not in _CACHE:
        p = np.arange(128)
        d16 = (p[:, None] % 16 == p[None, :] % 16).astype(np.float16)
        d32 = d16.astype(np.float32)
        dout = (p[:, None] % 16 == np.arange(BC)[None, :]).astype(np.float16)
        bc16 = (np.arange(BC)[:, None] == p[None, :] % 16).astype(np.float16)
        _CACHE["const"] = (d16, d32, dout, bc16)
    return _CACHE["const"]


def _prep_w(W):
    W5 = np.ascontiguousarray(W.reshape(R, O, DO, DI))
    # wt[8j+di, cc, do, o] = W[8cc+j, o, do, di]
    wt = np.ascontiguousarray(
        W5.reshape(CC, J, O, DO, DI).transpose(1, 4, 0, 3, 2)
    ).reshape(64, CC, DO, O).astype(np.float16)
    # ws[8rr+di, g, do, o] = W[16g+rr, o, do, di]
    ws = np.ascontiguousarray(
        W5.reshape(G0, 16, O, DO, DI).transpose(1, 4, 0, 3, 2)
    ).reshape(128, G0, DO, O).astype(np.float16)
    return wt, ws


def kernel(x: np.ndarray, W: np.ndarray) -> np.ndarray:
    x = np.asarray(x, dtype=np.float32)
    W = np.asarray(W, dtype=np.float32)
    nc = _get_nc()
    d16, d32, dout, bc16 = _prep_const()
    wt, ws = _prep_w(W)
    in_maps = []
    for q in range(NCORES):
        xq = x[BC * q : BC * (q + 1)]           # [16, 1152, 8]
        # xd[8j+di, cc, 16j'+b] = x[b, 8cc+j, di] * (j == j')
        xf = xq.reshape(BC, CC, J, DI).transpose(2, 3, 1, 0)  # [j, di, cc, b]
        xd = np.zeros((J, DI, CC, J, BC), dtype=np.float16)
        for j in range(J):
            xd[j, :, :, j, :] = xf[j]
        xd = np.ascontiguousarray(xd).reshape(64, CC, 128)
        # xs[8rr+di, g, b] = x[b, 16g+rr, di]
        xs = np.ascontiguousarray(
            xq.reshape(BC, G0, 16, DI).transpose(2, 3, 1, 0)
        ).reshape(128, G0, BC).astype(np.float16)
        in_maps.append({
            "wt": wt, "xd": xd, "xs": xs, "ws": ws, "bc16": bc16,
            "d16": d16, "d32": d32, "dout": dout,
        })
    res = run_bass_kernel_spmd(nc, in_maps, core_ids=list(range(NCORES)))
    out = np.concatenate([res.results[q]["out"] for q in range(NCORES)], axis=0)
    return out.reshape(B, O, DO, 1).astype(np.float32)


# revision 47
# speedup vs baseline: 1.0724x; 1.0724x over previous
"""DigitCapsule (dynamic routing) Trainium2 Bass kernel.

Problem: x (128,1152,8) f32, W (1,1152,10,16,8) f32 ->
  u_hat[b,r,o,do] = sum_di W[r,o,do,di] x[b,r,di]
  3 routing iterations (softmax over routes r, squash), output v (128,10,16,1).

Sharding: data-parallel over batch, 16 samples per core, W replicated.

Per-core layout (partition p = 16*j + b, j = r mod 8, b = batch-in-core):
  u[p, cc, do, o] = u_hat[b, 8*cc+j, o, do]   (fp16, 144 x 16 x 10 free)

Key structure:
  - xd (block-diag x stationary) built on host incl. zeros -> plain DMA
    (input DMA 5.4 MB total; the DMA stream paces the production phase).
  - u produced by 144 matmuls; PSUM->SBUF eviction alternates DVE/Act.
  - s0 = sum_r u via the delta-matrix chain (d16) interleaved into the
    production stream with a 2-batch lag (PE is in-order).
  - squash is elementwise: v = s*|s|/(1+s^2)  (mag_sq in the reference is
    over the trailing singleton axis).  Only Exp/Abs/Square activation
    functions are used -> a single LoadActFuncSet.
  - agreement premul+tree all-fp16 (2x DVE mode); the Pool engine owns
    group 0's premul+tree end to end (own tile buffers so it never blocks
    the in-order DVE stream).
  - exp per group on Act, overlapped with the agreement trees.
  - s-chain: c16 and premuls on DVE (Pool does group 0's premul); the
    psum accumulation consumes Pool's group mid-chain; dummy matmuls
    pre-warm the PE p-state during the softmax window.
  - input DMAs are chunked and emitted interleaved with the production
    batches (DMA completion sems are cumulative per queue, so emitting
    them all up front would gate the first matmul on the last chunk).
"""

import numpy as np

import concourse.bacc as bacc
import concourse.bass as bass
import concourse.tile as tile
from concourse import mybir
from concourse.bass_utils import run_bass_kernel_spmd

B, R, O, DO, DI = 128, 1152, 10, 16, 8
NCORES = 8
BC = B // NCORES          # 16 samples per core
J = 8                     # routes per matmul group
CC = R // J               # 144 matmul groups
OD = O * DO               # 160
G0 = 72                   # k=128 chunks for direct s0 (16 routes x 8 di)
F16 = mybir.dt.float16
F32 = mybir.dt.float32
AF = mybir.ActivationFunctionType
ALU = mybir.AluOpType

PROD_BATCH = 2            # cc per production psum batch (1 bank each)
TREE_BATCH = 24           # cc per premult/tree batch
NG = CC // TREE_BATCH     # 6 groups
POOL_GROUP = 0            # premul/tree group owned by the Pool engine
N_WARM = 3                # PE warm-up dummy matmuls per routing iteration


def _tl(pool, shape, tag):
    tile_h = pool.tile(shape, F16, tag=tag, name=tag)
    return tile_h


def _squash_elem(nc, pool, s_ps, v_out, scale, tag):
    """v_out = squash(s_ps * scale) elementwise: v = k2*s*|s| / (1 + (k*s)^2)."""
    P = s_ps.shape[0]
    q = pool.tile([P, DO, O], F32, tag=tag + "q")
    ab = pool.tile([P, DO, O], F32, tag=tag + "a")
    d = pool.tile([P, DO, O], F32, tag=tag + "d")
    p1 = pool.tile([P, DO, O], F32, tag=tag + "p")
    nc.scalar.activation(q[:], s_ps[:], AF.Square, scale=float(scale))
    nc.scalar.activation(ab[:], s_ps[:], AF.Abs, scale=float(scale * scale))
    nc.vector.tensor_scalar_add(d[:], q[:], 1.0)
    nc.vector.reciprocal(d[:], d[:])
    nc.vector.tensor_mul(p1[:], s_ps[:], ab[:])
    nc.vector.tensor_mul(v_out[:], p1[:], d[:])


def build_nc():
    nc = bacc.Bacc("TRN2", debug=False)
    wt_d = nc.dram_tensor("wt", [64, CC, DO, O], F16, kind="ExternalInput")
    xd_d = nc.dram_tensor("xd", [64, CC, 128], F16, kind="ExternalInput")
    xs_d = nc.dram_tensor("xs", [128, 72, BC], F16, kind="ExternalInput")
    ws_d = nc.dram_tensor("ws", [128, 72, DO, O], F16, kind="ExternalInput")
    bc16_d = nc.dram_tensor("bc16", [BC, 128], F16, kind="ExternalInput")
    d16_d = nc.dram_tensor("d16", [128, 128], F16, kind="ExternalInput")
    d32_d = nc.dram_tensor("d32", [128, 128], F32, kind="ExternalInput")
    dout_d = nc.dram_tensor("dout", [128, BC], F16, kind="ExternalInput")
    out_d = nc.dram_tensor("out", [BC, O, DO], F32, kind="ExternalOutput")

    with tile.TileContext(nc) as tc:
        with (
            tc.tile_pool(name="const", bufs=1) as const,
            tc.tile_pool(name="prod", bufs=1) as prod,
            tc.tile_pool(name="main", bufs=1) as main,
            tc.tile_pool(name="sq", bufs=1) as sq,
            tc.tile_pool(name="tp", bufs=3) as tp,
            tc.tile_pool(name="l1p", bufs=2) as l1p,
            tc.tile_pool(name="l2p", bufs=2) as l2p,
            tc.tile_pool(name="l3p", bufs=2) as l3p,
            tc.tile_pool(name="l4p", bufs=2) as l4p,
            tc.tile_pool(name="pb", bufs=1) as pb,
            tc.tile_pool(name="pp", bufs=3, space=bass.MemorySpace.PSUM) as pp,
            tc.tile_pool(name="pss", bufs=1, space=bass.MemorySpace.PSUM) as pss,
            tc.tile_pool(name="psd", bufs=1, space=bass.MemorySpace.PSUM) as psd,
        ):
            d16 = const.tile([128, 128], F16)
            d32 = const.tile([128, 128], F32)
            dout = const.tile([128, BC], F16)
            bc16 = const.tile([BC, 128], F16)
            nc.sync.dma_start(d16[:], d16_d[:])
            nc.sync.dma_start(d32[:], d32_d[:])
            nc.sync.dma_start(dout[:], dout_d[:])
            nc.sync.dma_start(bc16[:], bc16_d[:])
            zero = const.tile([128, 1], F32)
            ebias = const.tile([128, 1], F32)
            nc.vector.memset(zero[:], 0.0)
            nc.vector.memset(ebias[:], -13.8629436)

            NCH = 8
            cch = CC // NCH
            g0pc = G0 // NCH              # s0 k-chunks per DMA chunk (9)
            xd_t, wt_t, ws_t = [None] * NCH, [None] * NCH, [None] * NCH

            xs = prod.tile([128, G0, BC], F16)

            def fetch_chunk(ch):
                sl = slice(ch * cch, (ch + 1) * cch)
                xd_c = prod.tile([64, cch, 128], F16, tag=f"xd{ch % 3}",
                                 name="xd_c")
                wt_c = prod.tile([64, cch, DO, O], F16, tag=f"wt{ch % 3}",
                                 name="wt_c")
                ws_c = prod.tile([128, g0pc, DO, O], F16, tag=f"ws{ch % 3}",
                                 name="ws_c")
                sg = slice(ch * g0pc, (ch + 1) * g0pc)
                nc.sync.dma_start(ws_c[:], ws_d[:, sg, :, :])
                nc.sync.dma_start(xd_c[:], xd_d[:, sl, :])
                nc.sync.dma_start(wt_c[:], wt_d[:, sl, :, :])
                xd_t[ch] = xd_c
                wt_t[ch] = wt_c
                ws_t[ch] = ws_c

            fetch_chunk(0)
            nc.sync.dma_start(xs[:], xs_d[:])
            fetch_chunk(1)

            u = main.tile([128, CC, DO, O], F16)

            # ---- produce u_hat; s0 accumulates directly from (x, W) ----
            s0_ps = pss.tile([BC, DO, O], F32, tag="s")
            nb = CC // PROD_BATCH
            bpc = cch // PROD_BATCH       # batches per DMA chunk (9)
            v = main.tile([128, DO, O], F16)

            def emit_s0_chunk(cs):
                for k in range(g0pc):
                    g0 = cs * g0pc + k
                    nc.tensor.matmul(
                        s0_ps[:], xs[:, g0, :], ws_t[cs][:, k, :, :],
                        start=(g0 == 0), stop=(g0 == G0 - 1),
                    )

            for g in range(nb):
                ch = g // bpc
                if g % bpc == 0:
                    # prefetch two chunks ahead of consumption
                    if ch + 2 < NCH:
                        fetch_chunk(ch + 2)
                    # s0 matmuls run one segment ahead of their chunk's
                    # production so s0 (hence v0) completes before the
                    # eviction-paced production tail
                    if ch == 0:
                        emit_s0_chunk(0)
                        emit_s0_chunk(1)
                    elif ch + 1 < NCH:
                        emit_s0_chunk(ch + 1)
                    if ch == 6:
                        # v0 = squash(s0 / R) on 16 partitions + broadcast.
                        # All on DVE: the Act queue is full of tail
                        # evictions, and DVE is idle waiting for v anyway.
                        s0sb = sq.tile([BC, DO, O], F32, tag="s0s")
                        q0 = sq.tile([BC, DO, O], F32, tag="s0q")
                        a0 = sq.tile([BC, DO, O], F32, tag="s0a")
                        v0_16 = main.tile([BC, DO, O], F16)
                        k = 1.0 / R
                        nc.vector.tensor_scalar_mul(s0sb[:], s0_ps[:], k)
                        nc.vector.tensor_mul(q0[:], s0sb[:], s0sb[:])
                        nc.vector.scalar_tensor_tensor(
                            a0[:], s0sb[:], -1.0, s0sb[:],
                            ALU.mult, ALU.max)
                        nc.vector.tensor_scalar_add(q0[:], q0[:], 1.0)
                        nc.vector.reciprocal(q0[:], q0[:])
                        nc.vector.tensor_mul(a0[:], a0[:], s0sb[:])
                        nc.vector.tensor_mul(a0[:], a0[:], q0[:])
                        nc.vector.tensor_copy(v0_16[:], a0[:])
                        v_ps = pss.tile([128, DO, O], F32, tag="s")
                        nc.tensor.matmul(v_ps[:], bc16[:], v0_16[:],
                                         start=True, stop=True)
                        nc.vector.tensor_copy(v[:], v_ps[:])
                ps = pp.tile([128, PROD_BATCH, 512], F32, tag="pp")
                for i in range(PROD_BATCH):
                    cc = g * PROD_BATCH + i
                    ci = cc % cch
                    nc.tensor.matmul(
                        ps[:, i, 0:OD], xd_t[ch][:, ci, :],
                        wt_t[ch][:, ci, :, :],
                        start=True, stop=True,
                    )
                sl = slice(g * PROD_BATCH, (g + 1) * PROD_BATCH)
                src = ps[:, :, 0:OD].rearrange(
                    "p c (do o) -> p c do o", do=DO)
                # first 6 chunks alternate Act/DVE; tail on Act only so DVE
                # is free to start the iter-1 agreement as soon as v0 lands
                if ch >= 6 or g % 2 == 0:
                    nc.scalar.copy(u[:, sl, :, :], src)
                else:
                    nc.vector.tensor_copy(u[:, sl, :, :], src)

            b_ij = main.tile([128, CC, O], F32)
            e16 = main.tile([128, CC, O], F16)
            escr = main.tile([128, CC], F16)
            e_r = main.tile([128, O], F32)
            inv = main.tile([128, O], F32)

            for it in (1, 2):
                final = it == 2
                # ---- agreement: b_ij (+)= sum_do u * v ----
                l3_last = None
                for g in range(NG):
                    sl = slice(g * TREE_BATCH, (g + 1) * TREE_BATCH)
                    pool_g = g == POOL_GROUP
                    eng = nc.gpsimd if pool_g else nc.vector
                    sfx = "P" if pool_g else ""
                    if pool_g:
                        t = pb.tile([128, TREE_BATCH, DO, O], F16,
                                    tag="t0P", name="t")
                    else:
                        t = tp.tile([128, TREE_BATCH, DO, O], F16, tag="t")
                    v_b = v[:].unsqueeze(1).broadcast_to((128, TREE_BATCH, DO, O))
                    eng.tensor_mul(t[:], u[:, sl, :, :], v_b)
                    l1 = _tl(pb if pool_g else l1p, [128, TREE_BATCH, 8, O], "l1" + sfx)
                    eng.tensor_add(l1[:], t[:, :, 0:8, :], t[:, :, 8:16, :])
                    l2 = _tl(pb if pool_g else l2p, [128, TREE_BATCH, 4, O], "l2" + sfx)
                    eng.tensor_add(l2[:], l1[:, :, 0:4, :], l1[:, :, 4:8, :])
                    l3 = _tl(pb if pool_g else l3p, [128, TREE_BATCH, 2, O], "l3" + sfx)
                    eng.tensor_add(l3[:], l2[:, :, 0:2, :], l2[:, :, 2:4, :])
                    if it == 1:
                        eng.tensor_add(
                            b_ij[:, sl, :], l3[:, :, 0, :], l3[:, :, 1, :])
                    else:
                        a4 = _tl(pb if pool_g else l4p, [128, TREE_BATCH, O], "l4" + sfx)
                        eng.tensor_add(a4[:], l3[:, :, 0, :], l3[:, :, 1, :])
                        eng.tensor_add(b_ij[:, sl, :], b_ij[:, sl, :], a4[:])
                    if not pool_g:
                        l3_last = l3
                    # exp of this group overlaps the next group's tree
                    # (Act).  fp16 with a fixed bias shift: b1 in [-3, 3],
                    # b2 max ~24 -> shift by 20*ln2 keeps it in fp16 range;
                    # the softmax normalizer absorbs the shift exactly.
                    bias = zero if it == 1 else ebias
                    nc.scalar.activation(e16[:, sl, :], b_ij[:, sl, :],
                                         AF.Exp, bias=bias[:])

                # PE p-state warm-up while the premuls restart the stream
                warm = psd.tile([128, 512], F32, tag="den")
                for w in range(N_WARM):
                    nc.tensor.matmul(
                        warm[:, 0:480],
                        d16[:], l3_last[:].rearrange("p c x o -> p (c x o)"),
                        start=True, stop=True)

                # ---- s_raw = sum_r e * u  (normalizer applied post-hoc,
                # so premuls don't wait for the denominator) ----
                sp_p = BC if final else 128
                lhs = dout if final else d16
                s_ps2 = pss.tile([sp_p, DO, O], F32, tag="s")
                order = [POOL_GROUP] + [g for g in range(NG) if g != POOL_GROUP]
                mm_order = [1, 2, 3, POOL_GROUP, 4, 5]
                t_tiles = {}
                for k, g in enumerate(order):
                    sl = slice(g * TREE_BATCH, (g + 1) * TREE_BATCH)
                    pool_g = g == POOL_GROUP
                    eng = nc.gpsimd if pool_g else nc.vector
                    if pool_g:
                        t = pb.tile([128, TREE_BATCH, DO, O], F16, tag="t0P")
                    else:
                        t = tp.tile([128, TREE_BATCH, DO, O], F16, tag="t")
                    c_b = e16[:, sl, :].unsqueeze(2).broadcast_to(
                        (128, TREE_BATCH, DO, O))
                    if g == NG - 1:
                        # halves: matmuls on the first half overlap the
                        # second half's premul, shrinking the phase tail
                        hb = TREE_BATCH // 2
                        c_b1 = e16[:, sl, :][:, 0:hb, :].unsqueeze(2).broadcast_to(
                            (128, hb, DO, O))
                        c_b2 = e16[:, sl, :][:, hb:, :].unsqueeze(2).broadcast_to(
                            (128, hb, DO, O))
                        eng.tensor_mul(t[:, 0:hb, :, :],
                                       u[:, sl, :, :][:, 0:hb, :, :], c_b1)
                        for i in range(hb):
                            nc.tensor.matmul(
                                s_ps2[:], lhs[:, :sp_p], t[:, i, :, :],
                                start=False, stop=False)
                        eng.tensor_mul(t[:, hb:, :, :],
                                       u[:, sl, :, :][:, hb:, :, :], c_b2)
                        for i in range(hb, TREE_BATCH):
                            nc.tensor.matmul(
                                s_ps2[:], lhs[:, :sp_p], t[:, i, :, :],
                                start=False, stop=(i == TREE_BATCH - 1))
                        t_tiles[g] = t
                        continue
                    eng.tensor_mul(t[:], u[:, sl, :, :], c_b)
                    t_tiles[g] = t
                    if pool_g:
                        continue
                    ki = mm_order.index(g)
                    for i in range(TREE_BATCH):
                        nc.tensor.matmul(
                            s_ps2[:], lhs[:, :sp_p], t[:, i, :, :],
                            start=(ki == 0 and i == 0),
                            stop=False,
                        )
                    if ki + 1 < NG and mm_order[ki + 1] == POOL_GROUP:
                        tpg = t_tiles[POOL_GROUP]
                        for i in range(TREE_BATCH):
                            nc.tensor.matmul(
                                s_ps2[:], lhs[:, :sp_p], tpg[:, i, :, :],
                                start=False, stop=False,
                            )
                # ---- softmax denominator: 10 per-o accumulate-copies on
                # the otherwise-idle Act engine (frees DVE of the reduce) ----
                for o in range(O):
                    nc.scalar.activation(
                        escr[:], e16[:, :, o], AF.Copy,
                        accum_out=e_r[:, o : o + 1])
                den = psd.tile([128, 512], F32, tag="den")
                nc.tensor.matmul(den[:, 0:O], d32[:], e_r[:], start=True, stop=True)
                nc.vector.reciprocal(inv[:], den[:, 0:O])
                # ---- s = s_raw * inv, then squash ----
                sx = sq.tile([sp_p, DO, O], F32, tag="sx", name="sx")
                ivb = inv[0:sp_p, :].unsqueeze(1).broadcast_to((sp_p, DO, O))
                nc.vector.tensor_mul(sx[:], s_ps2[:], ivb)
                if not final:
                    _squash_elem(nc, sq, sx, v, 1.0, tag="sv")
                else:
                    v2 = main.tile([BC, DO, O], F32)
                    _squash_elem(nc, sq, sx, v2, 1.0, tag="sf")
                    v2p = main.tile([BC, O, DO], F32)
                    nc.vector.tensor_copy(v2p[:], v2[:].transpose((0, 2, 1)))
                    nc.sync.dma_start(out_d[:], v2p[:])

    nc.compile()
    return nc


_CACHE = {}


def _get_nc():
    if "nc" not in _CACHE:
        _CACHE["nc"] = build_nc()
    return _CACHE["nc"]


def _prep_const():
    if "const" not in _CACHE:
        p = np.arange(128)
        d16 = (p[:, None] % 16 == p[None, :] % 16).astype(np.float16)
        d32 = d16.astype(np.float32)
        dout = (p[:, None] % 16 == np.arange(BC)[None, :]).astype(np.float16)
        bc16 = (np.arange(BC)[:, None] == p[None, :] % 16).astype(np.float16)
        _CACHE["const"] = (d16, d32, dout, bc16)
    return _CACHE["const"]


def _prep_w(W):
    W5 = np.ascontiguousarray(W.reshape(R, O, DO, DI))
    # wt[8j+di, cc, do, o] = W[8cc+j, o, do, di]
    wt = np.ascontiguousarray(
        W5.reshape(CC, J, O, DO, DI).transpose(1, 4, 0, 3, 2)
    ).reshape(64, CC, DO, O).astype(np.float16)
    # ws[8rr+di, g, do, o] = W[16g+rr, o, do, di]
    ws = np.ascontiguousarray(
        W5.reshape(G0, 16, O, DO, DI).transpose(1, 4, 0, 3, 2)
    ).reshape(128, G0, DO, O).astype(np.float16)
    return wt, ws


def kernel(x: np.ndarray, W: np.ndarray) -> np.ndarray:
    x = np.asarray(x, dtype=np.float32)
    W = np.asarray(W, dtype=np.float32)
    nc = _get_nc()
    d16, d32, dout, bc16 = _prep_const()
    wt, ws = _prep_w(W)
    in_maps = []
    for q in range(NCORES):
        xq = x[BC * q : BC * (q + 1)]           # [16, 1152, 8]
        # xd[8j+di, cc, 16j'+b] = x[b, 8cc+j, di] * (j == j')
        xf = xq.reshape(BC, CC, J, DI).transpose(2, 3, 1, 0)  # [j, di, cc, b]
        xd = np.zeros((J, DI, CC, J, BC), dtype=np.float16)
        for j in range(J):
            xd[j, :, :, j, :] = xf[j]
        xd = np.ascontiguousarray(xd).reshape(64, CC, 128)
        # xs[8rr+di, g, b] = x[b, 16g+rr, di]
        xs = np.ascontiguousarray(
            xq.reshape(BC, G0, 16, DI).transpose(2, 3, 1, 0)
        ).reshape(128, G0, BC).astype(np.float16)
        in_maps.append({
            "wt": wt, "xd": xd, "xs": xs, "ws": ws, "bc16": bc16,
            "d16": d16, "d32": d32, "dout": dout,
        })
    res = run_bass_kernel_spmd(nc, in_maps, core_ids=list(range(NCORES)))
    out = np.concatenate([res.results[q]["out"] for q in range(NCORES)], axis=0)
    return out.reshape(B, O, DO, 1).astype(np.float32)
